# revision 10
# baseline (speedup 1.0000x reference)
"""Llama GQA causal attention layer (the "topk" in the module name is a
mathematical identity) on 8 Trainium2 NeuronCores.

Sharding: tensor-parallel over heads. Each core owns 2 of the 16 q-heads and
the single kv-head they share, computes its slice of Q/K/V projections, RoPE,
causal flash attention (scores kept on-chip in [k, q] orientation), and a
row-slice o_proj producing a full-shape [S, HID] partial; the host sums the 8
partials (the o_proj row-parallel AllReduce done on host).

Shapes hardcoded per problem spec:
  hidden_states [1, 4096, 2048] f32, position_ids [1, 4096] i32,
  Wq [2048, 2048], Wk/Wv [2048, 512], Wo [2048, 2048] f32.
"""

import math
import os
import sys

import numpy as np

if "/opt/trn_rl_repo" not in sys.path:
    sys.path.insert(0, "/opt/trn_rl_repo")

import concourse.bass as bass
import concourse.mybir as mybir
import concourse.tile as tile
from concourse import bacc, bass_utils

B, S, HID = 1, 4096, 2048
NH, KVH, HD = 16, 4, 128
GROUPS = NH // KVH
NCORES = 8
HPC = NH // NCORES          # q heads per core = 2
ST = S // 128               # 32 s-tiles
KT = HID // 128             # 16 hid-tiles (contraction)
QCH = 512                   # q chunk width for attention
NQC = S // QCH
ROPE_THETA = 10000.0
ISQ = 1.0 / math.sqrt(HD)

F32 = mybir.dt.float32
F32R = mybir.dt.float32r

# matmul operand dtype: float32r streams at 1 cyc/row (vs 4 for float32) when
# the moving dim >= 256; storage bytes are identical to f32 so we bitcast APs
# at the matmul callsites only.
USE_F32R = os.environ.get("KERNEL_MM_DT", "f32r") == "f32r"


def _r(ap):
    return ap


def build_body(tc, out, ht, wqkv, wo, cosr, sinr, identd, trimaskd, onesd):
    """Emit the per-core program.

    DRAM layouts (host pre-arranged, partition dim first):
      ht    [128, KT, S]   ht[p, t, s]  = hidden[s, 128 t + p]
      wqkv  [128, KT, 512] wqkv[p,t,j]  = [Wq_c | Wk_c | Wv_c][128 t + p, j]
      wo    [128, HPC, HID] wo[p, j, n] = Wo[256 c + 128 j + p, n]
      cosr/sinr [128, ST, 64] cosr[p, st, f] = cos[128 st + p, f]
      out   [S, HID] partial output (sum over cores on host)
    """
    nc = tc.nc
    exitstack = []

    with (
        tc.tile_pool(name="const", bufs=1) as constp,
        tc.tile_pool(name="slabs", bufs=1) as slabs,
    ):
        # Resident constants
        wqkv_sb = constp.tile([128, KT, 512], F32R)
        nc.sync.dma_start(out=wqkv_sb, in_=wqkv)
        cos_sb = constp.tile([128, ST, 64], F32)
        nc.sync.dma_start(out=cos_sb, in_=cosr)
        sin_sb = constp.tile([128, ST, 64], F32)
        nc.sync.dma_start(out=sin_sb, in_=sinr)
        wo_sb = constp.tile([128, HPC, HID], F32R)
        nc.sync.dma_start(out=wo_sb, in_=wo)
        ident = constp.tile([128, 128], F32R)
        nc.sync.dma_start(out=ident, in_=identd)
        # trimask[ki, qi] = 1.0 where qi >= ki else 0 (valid causal, k-major layout)
        trimask = constp.tile([128, 128], F32R)
        nc.sync.dma_start(out=trimask, in_=trimaskd)
        ones_col = constp.tile([128, 1], F32R)
        nc.sync.dma_start(out=ones_col, in_=onesd[:, 0:1])
        ones_row = constp.tile([1, 128], F32)
        nc.sync.dma_start(out=ones_row, in_=onesd[0:1, :].bitcast(F32))

        # Resident activation slabs
        qT = slabs.tile([128, HPC, S], F32R)    # rotated Q^T  [hd, h, s]
        kTs = slabs.tile([128, S], F32R)        # rotated K^T  [hd, s]
        vsb = slabs.tile([128, ST, 128], F32R)  # V            [k%128, ktile, hd]
        attT = slabs.tile([128, HPC, S], F32R)  # attn out^T (softmax-scaled)

        # ---- Stage A: QKV projection + RoPE + transposes -------------------
        with (
            tc.tile_pool(name="a_sb", bufs=3) as ap_,
            tc.tile_pool(name="a_ps", bufs=2, space="PSUM") as apsum,
            tc.tile_pool(name="a_pt", bufs=3, space="PSUM") as atp,
        ):
            for st in range(ST):
                s0 = st * 128
                hs = ap_.tile([128, KT, 128], F32R, tag="hs")
                nc.sync.dma_start(out=hs, in_=ht[:, :, s0 : s0 + 128])
                pq = apsum.tile([128, 512], F32, tag="pqkv")
                for t in range(KT):
                    nc.tensor.matmul(
                        pq,
                        lhsT=_r(hs[:, t, :]),
                        rhs=_r(wqkv_sb[:, t, :]),
                        start=(t == 0),
                        stop=(t == KT - 1),
                    )
                pqv = pq.rearrange("p (j d) -> p j d", d=128)
                c = cos_sb[:, st, :]
                s_ = sin_sb[:, st, :]
                rot = ap_.tile([128, 3, 128], F32R, tag="rot")
                t1 = ap_.tile([128, 3, 64], F32, tag="t1")
                t2 = ap_.tile([128, 3, 64], F32, tag="t2")
                # q' / k' rotate-half (free-dim halves; cos/sin broadcast over j)
                for j in range(3):
                    a = pqv[:, j, 0:64]
                    b = pqv[:, j, 64:128]
                    nc.vector.tensor_mul(t1[:, j, :], a, c)
                    nc.vector.tensor_mul(t2[:, j, :], b, s_)
                    nc.vector.tensor_sub(rot[:, j, 0:64], t1[:, j, :], t2[:, j, :])
                    nc.vector.tensor_mul(t1[:, j, :], b, c)
                    nc.vector.tensor_mul(t2[:, j, :], a, s_)
                    nc.vector.tensor_add(rot[:, j, 64:128], t1[:, j, :], t2[:, j, :])
                # V copy out of PSUM
                nc.scalar.copy(vsb[:, st, :], pqv[:, 3, :])
                # transposes to [hd, s] layout
                for j, dest in (
                    (0, qT[:, 0, s0 : s0 + 128]),
                    (1, qT[:, 1, s0 : s0 + 128]),
                    (2, kTs[:, s0 : s0 + 128]),
                ):
                    pt = atp.tile([128, 128], F32R, tag="pt")
                    nc.tensor.transpose(pt, rot[:, j, :], ident)
                    if j == 2:
                        nc.vector.tensor_copy(dest, pt)
                    else:
                        nc.scalar.copy(dest, pt)

        # ---- Stage B: causal flash attention, scores^T [k, q] --------------
        with (
            tc.tile_pool(name="b_sb", bufs=4) as bp,
            tc.tile_pool(name="b_ps_s", bufs=2, space="PSUM") as bps,
            tc.tile_pool(name="b_ps_a", bufs=2, space="PSUM") as bpa,
            tc.tile_pool(name="b_ps_d", bufs=2, space="PSUM") as bpd,
        ):
            for h in range(HPC):
                for qc in range(NQC):
                    q0 = qc * QCH
                    att_ps = bpa.tile([128, QCH], F32, tag="attps")
                    den_ps = bpd.tile([1, QCH], F32, tag="denps")
                    nkt = q0 // 128 + QCH // 128
                    for kt in range(nkt):
                        k0 = kt * 128
                        off = max(0, k0 - q0)
                        ps = bps.tile([128, QCH], F32, tag="sc")
                        nc.tensor.matmul(
                            ps[:, off:],
                            lhsT=_r(kTs[:, k0 : k0 + 128]),
                            rhs=_r(qT[:, h, q0 + off : q0 + QCH]),
                            start=True,
                            stop=True,
                        )
                        pt_sb = bp.tile([128, QCH], F32R, tag="pT")
                        nc.scalar.activation(
                            pt_sb[:, off:],
                            ps[:, off:],
                            mybir.ActivationFunctionType.Exp,
                            scale=ISQ,
                        )
                        if k0 >= q0:  # diagonal block: zero strictly-future q
                            nc.vector.tensor_mul(
                                pt_sb[:, off : off + 128],
                                pt_sb[:, off : off + 128],
                                trimask,
                            )
                        nc.tensor.matmul(
                            att_ps[:, off:],
                            lhsT=_r(vsb[:, kt, :]),
                            rhs=_r(pt_sb[:, off:]),
                            start=(kt == 0),
                            stop=(kt == nkt - 1),
                        )
                        nc.tensor.matmul(
                            den_ps[:, off:],
                            lhsT=_r(ones_col),
                            rhs=_r(pt_sb[:, off:]),
                            start=(kt == 0),
                            stop=(kt == nkt - 1),
                        )
                    den_sb = bp.tile([1, QCH], F32, tag="den")
                    nc.vector.tensor_copy(den_sb, den_ps)
                    rden = bp.tile([1, QCH], F32, tag="rden")
                    nc.vector.reciprocal(rden, den_sb)
                    # broadcast 1/den across partitions via rank-1 matmul,
                    # stage through SBUF (DVE has a single PSUM read port)
                    rdb_ps = bps.tile([128, QCH], F32, tag="rdbps")
                    nc.tensor.matmul(
                        rdb_ps, lhsT=ones_row, rhs=rden, start=True, stop=True
                    )
                    rdb = bp.tile([128, QCH], F32, tag="rdb")
                    nc.scalar.copy(rdb, rdb_ps)
                    nc.vector.tensor_mul(attT[:, h, q0 : q0 + QCH], att_ps, rdb)

        # ---- Stage C: o_proj (row-parallel partial) ------------------------
        with (
            tc.tile_pool(name="c_sb", bufs=3) as cp,
            tc.tile_pool(name="c_ps", bufs=4, space="PSUM") as cps,
        ):
            for st in range(ST):
                s0 = st * 128
                osb = cp.tile([128, HID], F32, tag="osb")
                for nch in range(HID // 512):
                    n0 = nch * 512
                    po = cps.tile([128, 512], F32, tag="po")
                    for j in range(HPC):
                        nc.tensor.matmul(
                            po,
                            lhsT=_r(attT[:, j, s0 : s0 + 128]),
                            rhs=_r(wo_sb[:, j, n0 : n0 + 512]),
                            start=(j == 0),
                            stop=(j == HPC - 1),
                        )
                    if nch % 2 == 0:
                        nc.scalar.copy(osb[:, n0 : n0 + 512], po)
                    else:
                        nc.vector.tensor_copy(osb[:, n0 : n0 + 512], po)
                nc.sync.dma_start(out=out[s0 : s0 + 128, :], in_=osb)


_NC_CACHE = {}


def get_nc():
    key = "nc"
    if key not in _NC_CACHE:
        nc = bacc.Bacc(
            "TRN2",
            debug=False,
            enable_asserts=False,
            target_bir_lowering=False,
        )
        ht = nc.dram_tensor("ht", [128, KT, S], F32R, kind="ExternalInput").ap()
        wqkv = nc.dram_tensor("wqkv", [128, KT, 512], F32R, kind="ExternalInput").ap()
        wo = nc.dram_tensor("wo", [128, HPC, HID], F32R, kind="ExternalInput").ap()
        cosr = nc.dram_tensor("cosr", [128, ST, 64], F32, kind="ExternalInput").ap()
        sinr = nc.dram_tensor("sinr", [128, ST, 64], F32, kind="ExternalInput").ap()
        identd = nc.dram_tensor("identd", [128, 128], F32R, kind="ExternalInput").ap()
        trimaskd = nc.dram_tensor("trimaskd", [128, 128], F32R, kind="ExternalInput").ap()
        onesd = nc.dram_tensor("onesd", [128, 128], F32R, kind="ExternalInput").ap()
        out = nc.dram_tensor("out", [S, HID], F32, kind="ExternalOutput").ap()
        with tile.TileContext(nc) as tc:
            build_body(tc, out, ht, wqkv, wo, cosr, sinr, identd, trimaskd, onesd)
        nc.compile()
        _NC_CACHE[key] = nc
    return _NC_CACHE[key]


def prep_in_maps(hidden_states, position_ids, Wq, Wk, Wv, Wo):
    hid = np.asarray(hidden_states, dtype=np.float32)[0]          # [S, HID]
    pos = np.asarray(position_ids)[0].astype(np.float32)          # [S]
    Wq = np.asarray(Wq, dtype=np.float32)
    Wk = np.asarray(Wk, dtype=np.float32)
    Wv = np.asarray(Wv, dtype=np.float32)
    Wo = np.asarray(Wo, dtype=np.float32)

    inv = 1.0 / (ROPE_THETA ** (np.arange(0, HD, 2, dtype=np.float32) / HD))
    freqs = pos[:, None] * inv[None, :]                           # [S, 64]
    cos_r = np.ascontiguousarray(
        np.cos(freqs).astype(np.float32).reshape(ST, 128, 64).transpose(1, 0, 2)
    )
    sin_r = np.ascontiguousarray(
        np.sin(freqs).astype(np.float32).reshape(ST, 128, 64).transpose(1, 0, 2)
    )
    ht_r = np.ascontiguousarray(hid.T.reshape(KT, 128, S).transpose(1, 0, 2))

    in_maps = []
    for c in range(NCORES):
        kv = c // 2
        wqkv_c = np.concatenate(
            [
                Wq[:, 256 * c : 256 * (c + 1)],
                Wk[:, 128 * kv : 128 * (kv + 1)],
                Wv[:, 128 * kv : 128 * (kv + 1)],
            ],
            axis=1,
        )                                                          # [2048, 512]
        wqkv_r = np.ascontiguousarray(
            wqkv_c.reshape(KT, 128, 512).transpose(1, 0, 2)
        )
        wo_r = np.ascontiguousarray(
            Wo[256 * c : 256 * (c + 1), :].reshape(HPC, 128, HID).transpose(1, 0, 2)
        )
        in_maps.append(
            {
                "ht": ht_r,
                "wqkv": wqkv_r,
                "wo": wo_r,
                "cosr": cos_r,
                "sinr": sin_r,
                "identd": np.eye(128, dtype=np.float32),
                "trimaskd": np.triu(np.ones((128, 128), np.float32)),
                "onesd": np.ones((128, 128), np.float32),
            }
        )
    return in_maps


def run_spmd(in_maps, **kw):
    nc = get_nc()
    return bass_utils.run_bass_kernel_spmd(
        nc, in_maps, core_ids=list(range(NCORES)), **kw
    )


def kernel(hidden_states, position_ids, Wq, Wk, Wv, Wo):
    in_maps = prep_in_maps(hidden_states, position_ids, Wq, Wk, Wv, Wo)
    res = run_spmd(in_maps)
    total = res.results[0]["out"].astype(np.float32)
    for c in range(1, NCORES):
        total = total + res.results[c]["out"]
    return total[None]
